# revision 47
# baseline (speedup 1.0000x reference)
"""Fused attention kernel for Trainium2, SPMD over 8 NeuronCores.

Problem: nn_AttentionFusion (B=8, S1=S2=2048, D1=D2=512, F=256, fp32).

    Q = feat1 @ Wq + bq            [B,S1,F]
    K = feat2 @ Wk + bk            [B,S2,F]
    V = feat2 @ Wv + bv            [B,S2,F]
    A = softmax(Q K^T / sqrt(F))   [B,S1,S2]
    out = (A @ V) @ Wfc + bfc      [B,S1,F]

Sharding: pure data-parallel over batch - core i computes batch element i.

v10 notes:
  * Wfc folded into the V projection on the host: Wpv = Wv @ Wfc and
    bout = bv @ Wfc + bfc (A@(V@Wfc) == (A@V)@Wfc; A@(1*b) = b since
    attention rows sum to 1).  bout is added into the V' tiles during
    their PSUM drain, so the PV epilogue is just recip + scale + store.
  * All matmul operands staged bf16 on the host in the exact SBUF tiling.
    feat2 is staged quarter-major ([P, DC, 128] tiles): DMA sustains only
    ~60-110 GB/s per queue at startup, so phase A streams 128KB quarters
    round-robin over the sync and gpsimd queues and starts computing on
    the first quarter ~8us in; K^T accumulates straight from quarter
    tiles.  feat1 slab 0 rides the scalar queue behind the weights;
    slabs 1-3 (needed much later) join the sync/gpsimd ring.
  * The DMA-bound ramp is hidden by pulling super-block 0's Q projection
    and all 8 score groups INTO phase A (score group g only needs K of
    super-block g//2), roughly doubling the PE work per DMA byte while
    the queues are slow.  A short burst of dummy matmuls bridges the wait
    for quarter 0 so the HAM clock-gate (PE 1.2 -> 2.4 GHz after ~3.4us
    of sustained activity) latches early and never resets.
  * fp16 output (eps 4.9e-4, negligible vs the 2e-2 gate) halves store
    bytes; stores are split across the gpsimd and sync queues.
  * Phase B runs the remaining scores/PV software pipeline (PV blocks of
    super-block sc-1 interleave with score groups of sc so the PE streams
    while ACT drains exp()).

Per-core layouts: Q^T/K^T live as [f, s]; V' natural [s2, f] with a
ones-column at col F so the PV matmul's column F accumulates the softmax
denominator; scores^T is exp'd straight out of PSUM into bf16 P^T tiles
which are exactly the PV stationary operand.  Normalization is deferred:
(P@V')/denom == softmax(P)@V'.
"""

from contextlib import ExitStack

import numpy as np

import concourse.bacc as bacc
import concourse.bass as bass
import concourse.mybir as mybir
import concourse.tile as tile
from concourse.bass_utils import run_bass_kernel_spmd

# Problem sizes (hardcoded per the harness contract).
B = 8
S = 2048          # S1 == S2
D = 512           # D1 == D2
F = 256           # fusion dim
N_CORES = 8
P = 128           # partitions

DC = D // P       # 4 d-chunks
FC = F // P       # 2 f-chunks
NS = S // P       # 16 s-tiles
SUPER = 512       # s1/s2 super-block width
NSUP = S // SUPER # 4 super-blocks
NWARM = 14        # HAM warm-up dummy matmuls (N=128, ~107ns each cold)

FP32 = mybir.dt.float32
BF16 = mybir.dt.bfloat16
FP16 = mybir.dt.float16


def attention_body(ctx, tc, out, feat1T, f2q, Wq, Wk, Wpv, cpack):
    """Emit the per-core attention program.

    out: [S, F] fp16 DRAM.
    feat1T: [NSUP*P, DC, SUPER] bf16 DRAM (pre-transposed, slab-major: row
      sc*P+p holds feat1[d=c*128+p, s=sc*512 : (sc+1)*512] for c in 0..3).
    f2q: [NSUP*4*P, DC, P] bf16: feat2 transposed, quarter-major (row
      (4*sc+q)*P+p holds feat2[d=c*128+p, s-range of quarter q of sc]).
    Wq/Wk/Wpv: [P, DC, F] bf16 (Wpv = Wv@Wfc), row p chunk c = W[c*128+p, :].
    cpack: [P, 2*FC + F] fp32: cols 0:FC bq, FC:2FC bk, 2FC: bout broadcast.
    """
    nc = tc.nc
    Ident = mybir.ActivationFunctionType.Identity
    Exp = mybir.ActivationFunctionType.Exp
    scale = 1.0 / float(np.sqrt(F))

    consts = ctx.enter_context(tc.tile_pool(name="consts", bufs=1))
    persist = ctx.enter_context(tc.tile_pool(name="persist", bufs=1))

    kt_sb = persist.tile([P, FC, S], BF16)      # K^T  [f, s2]
    # V' padded to F+2 columns: col F is the softmax-denominator ones column;
    # col F+1 is dead padding (keeps the moving free dim even).
    v_sb = persist.tile([P, NS, F + 2], BF16)   # V' (+ ones col) [s2, f+2]

    # bf16 tile for the HAM warm-up dummies (memset directly, ready early).
    warm_bf = consts.tile([P, P], BF16)
    nc.gpsimd.memset(warm_bf[:], 1.0)
    ones_stage = consts.tile([P, NS, 2], FP32)
    nc.gpsimd.memset(ones_stage[:], 1.0)
    nc.vector.tensor_copy(v_sb[:, :, F:F + 2], ones_stage[:])

    featT = ctx.enter_context(tc.tile_pool(name="featT", bufs=3))
    fq_pool = ctx.enter_context(tc.tile_pool(name="fq", bufs=12))

    # Load ring: 16 feat2 quarters (phase A), then feat1 slabs 1-3 (phase B),
    # round-robin over the sync and gpsimd queues.
    schedule = [("q", f2q, i) for i in range(4 * NSUP)] + \
               [("s", feat1T, sc) for sc in range(1, NSUP)]
    loads = {}
    emitted = [0]

    def emit_load():
        # gpsimd first: its engine stream starts ~0.5us before sync's at
        # boot, so quarter 0's transfer begins that much earlier.  (Tested
        # and rejected: splitting the first quarters across both queues -
        # the cold-DMA era is latency-bound, not byte-bound, so halving the
        # transfer size does not halve its ~5us arrival latency.)
        kind, fr, idx = schedule[emitted[0]]
        eng = nc.gpsimd if emitted[0] % 2 == 0 else nc.sync
        if kind == "q":
            fT = fq_pool.tile([P, DC, P], BF16, tag="fq")
        else:
            fT = featT.tile([P, DC, SUPER], BF16, tag="fT")
        eng.dma_start(fT[:], fr[idx * P:(idx + 1) * P])
        loads[emitted[0]] = fT
        emitted[0] += 1

    def consume_slab(k):
        fT = loads.pop(k)
        if emitted[0] < len(schedule):
            emit_load()
        return fT

    PREFETCH = 6
    for _ in range(PREFETCH):
        emit_load()

    # Weights (bf16, pre-tiled [P, DC, F]) on the scalar queue in order of
    # first use, then feat1 slab 0 (consumed by Q(0) mid-phase-A).
    wpv_sb = consts.tile([P, DC, F], BF16)
    nc.scalar.dma_start(wpv_sb[:], Wpv)
    wk_sb = consts.tile([P, DC, F], BF16)
    nc.scalar.dma_start(wk_sb[:], Wk)
    cpk = consts.tile([P, 2 * FC + F], FP32)
    nc.scalar.dma_start(cpk[:], cpack)
    # wq and f1T0 (640KB, not needed before ~19us) are allocated here but
    # their DMA triggers are emitted after K(0)'s drain: the scalar engine
    # then cannot fire them until ~13.5us, keeping the congested cold-DMA
    # head era free for the critical feat2 quarters (the early era is a
    # SHARED bottleneck across queues).
    wq_sb = consts.tile([P, DC, F], BF16)
    f1T0 = featT.tile([P, DC, SUPER], BF16, tag="fT")
    bq_sb = cpk[:, 0:FC]
    bk_sb = cpk[:, FC:2 * FC]
    bout_bc = cpk[:, 2 * FC:]

    qt_pool = ctx.enter_context(tc.tile_pool(name="qt", bufs=2))
    pt_pool = ctx.enter_context(tc.tile_pool(name="pt", bufs=2))
    o_pool = ctx.enter_context(tc.tile_pool(name="o", bufs=3))

    # scores PSUM spans phase A (super-block 0) and phase B: 2 x 2 banks.
    ps_sc = ctx.enter_context(tc.tile_pool(name="ps_sc", bufs=2, space="PSUM"))

    def emit_score_group(pt, qt, g):
        """One scores^T group: s2-chunk pair (2g, 2g+1) accumulated into
        a 2-bank PSUM tile, exp'd (1024 cols) straight into pt."""
        s2c = 2 * g
        pss = ps_sc.tile([P, 2, SUPER], FP32, tag="ps_sc")
        for half in range(2):
            for fc in range(FC):
                nc.tensor.matmul(
                    pss[:, half, :],
                    kt_sb[:, fc, (s2c + half) * P:(s2c + half + 1) * P],
                    qt[:, fc, :],
                    start=(fc == 0), stop=(fc == FC - 1),
                )
        nc.scalar.activation(pt[:, s2c:s2c + 2, :], pss[:], Exp, scale=scale)

    def emit_q(ps_pool, f1T, qt):
        """Q^T projection of one feat1 super-block slab into qt."""
        for fc in range(FC):
            psq = ps_pool.tile([P, SUPER], FP32, tag="ps_t")
            for dc in range(DC):
                nc.tensor.matmul(
                    psq[:],
                    wq_sb[:, dc, fc * P:(fc + 1) * P],
                    f1T[:, dc, :],
                    start=(dc == 0), stop=(dc == DC - 1),
                )
            nc.scalar.activation(
                qt[:, fc, :], psq[:], Ident, bias=bq_sb[:, fc:fc + 1],
            )

    def emit_vprime_tile(ps_pool, fq, i):
        """V' row-tile i (global s2-tile index) from quarter tile fq."""
        psv = ps_pool.tile([P, SUPER], FP32, tag="ps_t")
        for dc in range(DC):
            nc.tensor.matmul(
                psv[:, 0:F], fq[:, dc, :], wpv_sb[:, dc, :],
                start=(dc == 0), stop=(dc == DC - 1),
            )
        nc.vector.tensor_add(v_sb[:, i, 0:F], psv[:, 0:F], bout_bc)

    # ------- phase A: feat2 -> V', K^T; super-block 0 scores woven in -------
    qt0 = qt_pool.tile([P, FC, SUPER], BF16, tag="qt")
    pt0 = pt_pool.tile([P, NS, SUPER], BF16, tag="pt")
    with ExitStack() as phA:
        psA_v = phA.enter_context(tc.tile_pool(name="psA_v", bufs=2, space="PSUM"))
        psA_k = phA.enter_context(tc.tile_pool(name="psA_k", bufs=2, space="PSUM"))

        # HAM warm-up: N=128 matmuls on the memset tile bridge the wait for
        # the first quarter tile so the clock-gate flips to 2.4 GHz before
        # real work starts (an idle gap > ~3.4us re-throttles it).
        psw = psA_v.tile([P, SUPER], FP32, tag="ps_t")
        for _ in range(NWARM):
            nc.tensor.matmul(psw[:, 0:P], warm_bf[:], warm_bf[:],
                             start=True, stop=True)

        # scores(0) groups woven into phase A: per super-block sc the groups
        # [g0(sc), g1(sc)) are legal (they only need K of super-blocks <= sc)
        # and Q(0).  Q(0) sits after V'K(2) so its slab (last on the slow
        # scalar queue) has time to land.
        sc_groups = {2: (0, 3), 3: (3, 8)}

        for sc in range(NSUP):
            quarts = []
            # V' on each quarter as it lands (light PE demand while the DMA
            # queues are still slow), then the K parts from the stashed tiles.
            for q in range(4):
                fq = consume_slab(4 * sc + q)
                quarts.append(fq)
                emit_vprime_tile(psA_v, fq, 4 * sc + q)
            psks = [psA_k.tile([P, SUPER], FP32, tag="ps_k", name=f"psk{fc}")
                    for fc in range(FC)]
            for fc in range(FC):
                for q in range(4):
                    for dc in range(DC):
                        nc.tensor.matmul(
                            psks[fc][:, q * P:(q + 1) * P],
                            wk_sb[:, dc, fc * P:(fc + 1) * P],
                            quarts[q][:, dc, :],
                            start=(dc == 0), stop=(dc == DC - 1),
                        )
                nc.scalar.activation(
                    kt_sb[:, fc, sc * SUPER:(sc + 1) * SUPER], psks[fc][:],
                    Ident, bias=bk_sb[:, fc:fc + 1],
                )
            if sc == 0:
                # Deferred bulk loads: these triggers sit after K(0)'s drain
                # in the scalar engine stream, so they fire ~13.5us in.
                nc.scalar.dma_start(wq_sb[:], Wq)
                nc.scalar.dma_start(f1T0[:], feat1T[0:P])
            if sc == 2:
                emit_q(psA_v, f1T0, qt0)
            g0, g1 = sc_groups.get(sc, (0, 0))
            for g in range(g0, g1):
                emit_score_group(pt0, qt0, g)

    # ---------------- phase B: remaining scores + PV pipeline ----------------
    with ExitStack() as phB:
        ps_misc = phB.enter_context(tc.tile_pool(name="ps_misc", bufs=2, space="PSUM"))
        ps_at = phB.enter_context(tc.tile_pool(name="ps_at", bufs=2, space="PSUM"))

        def emit_pv_block(sup, b, pt):
            """PV + normalize + store for one 128-row s1 block."""
            blk = sup * SUPER + b * P
            psa = ps_at.tile([P, F + 2], FP32, tag="ps_at")
            for s2c in range(NS):
                nc.tensor.matmul(
                    psa[:],
                    pt[:, s2c, b * P:(b + 1) * P],
                    v_sb[:, s2c, :],
                    start=(s2c == 0), stop=(s2c == NS - 1),
                )
            recip = o_pool.tile([P, 1], FP32, tag="recip")
            nc.vector.reciprocal(recip[:], psa[:, F:F + 1])
            o_sb = o_pool.tile([P, F], FP16, tag="osb")
            nc.vector.tensor_scalar_mul(o_sb[:], psa[:, 0:F], recip[:])
            H = P // 2
            nc.gpsimd.dma_start(out[blk:blk + H, :], o_sb[0:H])
            nc.sync.dma_start(out[blk + H:blk + P, :], o_sb[H:P])

        pt_prev = pt0
        for sc in range(1, NSUP):
            f1T = consume_slab(4 * NSUP + sc - 1)
            # PV block 0 of the previous super-block covers the Q drain
            # latency before the first score group can start.
            emit_pv_block(sc - 1, 0, pt_prev)
            qt = qt_pool.tile([P, FC, SUPER], BF16, tag="qt")
            emit_q(ps_misc, f1T, qt)
            pt = pt_pool.tile([P, NS, SUPER], BF16, tag="pt")
            for b in range(4):
                emit_score_group(pt, qt, 2 * b)
                emit_score_group(pt, qt, 2 * b + 1)
                if b < 3:
                    emit_pv_block(sc - 1, b + 1, pt_prev)
            pt_prev = pt
        for b in range(4):
            emit_pv_block(NSUP - 1, b, pt_prev)


def build_program():
    # Bacc (not raw Bass): its compile() legalizes semaphore waits to the
    # TRN2 one-wait-per-instruction constraint.
    nc = bacc.Bacc("TRN2", target_bir_lowering=False, debug=False)
    feat1T = nc.dram_tensor("feat1T", [NSUP * P, DC, SUPER], BF16,
                            kind="ExternalInput").ap()
    f2q = nc.dram_tensor("f2q", [NSUP * 4 * P, DC, P], BF16,
                         kind="ExternalInput").ap()
    Wq = nc.dram_tensor("Wq", [P, DC, F], BF16, kind="ExternalInput").ap()
    Wk = nc.dram_tensor("Wk", [P, DC, F], BF16, kind="ExternalInput").ap()
    Wpv = nc.dram_tensor("Wpv", [P, DC, F], BF16, kind="ExternalInput").ap()
    cpack = nc.dram_tensor("cpack", [P, 2 * FC + F], FP32,
                           kind="ExternalInput").ap()
    # fp16 output (eps 4.9e-4, negligible vs the 2e-2 gate) halves store
    # bytes and the tail-critical final store latency.
    out = nc.dram_tensor("out", [S, F], FP16, kind="ExternalOutput").ap()

    with tile.TileContext(nc) as tc, ExitStack() as ctx:
        attention_body(ctx, tc, out, feat1T, f2q, Wq, Wk, Wpv, cpack)
    nc.compile()
    return nc


def _tile_weight(w, bf16):
    """[D, F] -> [P, DC, F] bf16 (row p, chunk c = W[c*128+p, :])."""
    w = np.asarray(w, dtype=np.float32).astype(bf16)
    return np.ascontiguousarray(w.reshape(DC, P, F).transpose(1, 0, 2))


def _tile_featT(feat, bf16):
    """[S, D] fp32 -> transposed slab-major [NSUP*P, DC, SUPER] bf16."""
    fT = np.asarray(feat).astype(bf16).T                     # [D, S]
    fT = fT.reshape(DC, P, NSUP, SUPER).transpose(2, 1, 0, 3)  # [sc, p, c, s]
    return np.ascontiguousarray(fT.reshape(NSUP * P, DC, SUPER))


def _tile_featQ(feat, bf16):
    """[S, D] fp32 -> transposed quarter-major [NSUP*4*P, DC, P] bf16."""
    fT = np.asarray(feat).astype(bf16).T                     # [D, S]
    fT = fT.reshape(DC, P, NSUP, 4, P).transpose(2, 3, 1, 0, 4)
    return np.ascontiguousarray(fT.reshape(NSUP * 4 * P, DC, P))


def run(inputs, trace=False, trace_kwargs=None):
    """Shard over 8 cores, execute, gather. Returns (output, BassKernelResults)."""
    import ml_dtypes
    bf16 = ml_dtypes.bfloat16

    nc = build_program()
    # Host-side fusion of the fc projection into V (exact in fp64), and
    # bf16 tiled staging of all matmul operands.
    Wv = np.asarray(inputs["Wv"], dtype=np.float64)
    Wfc = np.asarray(inputs["Wfc"], dtype=np.float64)
    bv = np.asarray(inputs["bv"], dtype=np.float64)
    bfc = np.asarray(inputs["bfc"], dtype=np.float64)
    bout = (bv @ Wfc + bfc).astype(np.float32)
    cpack = np.zeros((P, 2 * FC + F), np.float32)
    cpack[:, 0:FC] = np.asarray(inputs["bq"], dtype=np.float32).reshape(FC, P).T
    cpack[:, FC:2 * FC] = np.asarray(inputs["bk"], dtype=np.float32).reshape(FC, P).T
    cpack[:, 2 * FC:] = bout[None, :]
    shared = {
        "Wq": _tile_weight(inputs["Wq"], bf16),
        "Wk": _tile_weight(inputs["Wk"], bf16),
        "Wpv": _tile_weight((Wv @ Wfc).astype(np.float32), bf16),
        "cpack": cpack,
    }
    feat1 = np.asarray(inputs["feat1"])
    feat2 = np.asarray(inputs["feat2"])
    in_maps = [
        {
            "feat1T": _tile_featT(feat1[i], bf16),
            "f2q": _tile_featQ(feat2[i], bf16),
            **shared,
        }
        for i in range(N_CORES)
    ]
    res = run_bass_kernel_spmd(
        nc, in_maps, core_ids=list(range(N_CORES)),
        trace=trace, **(trace_kwargs or {}),
    )
    out = np.stack([res.results[i]["out"].astype(np.float32)
                    for i in range(N_CORES)], axis=0)
    return out, res


def kernel(**inputs) -> np.ndarray:
    out, _ = run(inputs)
    return out
